# revision 1
# baseline (speedup 1.0000x reference)
"""Self-contained Trainium2 kernel: block-circulant FFT linear layer.

out = ifft(fft(x_blocks) * fft(W)).real summed over input blocks -- computed
as three PE matmul stages (real-FFT basis, per-frequency block matmul,
inverse real-FFT) with DVE 32x32 block-transposes as the inter-stage data
shuffles, SPMD over 8 NeuronCores (batch-sharded).

kernel(x, W): x [4096, 4096] f32, W [64, 64, 64] f32 -> [4096, 4096] f32.
"""
import numpy as np
import concourse.bass as bass
import concourse.bacc as bacc
import concourse.mybir as mybir
import concourse.tile as tile
from concourse.tile import add_dep_helper
from concourse.bass_utils import run_bass_kernel_spmd

N_CORES = 8
B, IN, OUT, BS = 4096, 4096, 4096, 64
BC = B // N_CORES            # 512 batch rows per core
NK = 32                      # bin tiles (tile 0 carries bins 0 and 32)
NA = 32                      # jpair / ipair tiles
F32 = mybir.dt.float32
F32R = mybir.dt.float32r
MM_DT = F32R   # matmul input dtype (float32r: 4x PE rate, rounded inputs)

# tunables
BW = 256                     # batch window (matmul free dim)
NH = BC // BW
IN_SPLIT = 2                 # HBM in/out DMAs per window
SHUF_MODE = "strided"
USE_BARRIER = False
LEVEL = 5  # 0:io 1:+fwd 2:+sh1 3:+mid 4:+sh2 5:+inv
SH_ENGINES = ("alt", "alt")     # "alt": even idx sync, odd idx scalar
IO_ENGINES = ("sync", "scalar")   # engines for (in, out) DMA issue


# ---------------- host-side constant matrices ----------------

def make_fmat():
    t = np.arange(BS)[:, None]
    c = np.arange(BS)[None, :]
    k = np.where(c <= 32, c, c - 32)
    ang = 2 * np.pi * k * t / BS
    F = np.where(c <= 32, np.cos(ang), np.sin(ang))
    bd = np.zeros((128, 128), np.float32)
    bd[:64, :64] = F
    bd[64:, 64:] = F
    return bd


def make_gmat():
    tau = np.arange(BS)[None, :]
    c = np.arange(BS)[:, None]
    k = np.where(c <= 32, c, c - 32)
    ang = 2 * np.pi * k * tau / BS
    base = np.where(c <= 32, np.cos(ang), np.sin(ang))
    scale = np.where((c % 32) == 0, 1.0 / BS, 2.0 / BS)
    G = base * scale
    bd = np.zeros((128, 128), np.float32)
    bd[:64, :64] = G
    bd[64:, 64:] = G
    return bd


def make_wmats(W):
    s = np.arange(BS)
    k = np.arange(33)
    ang = 2 * np.pi * k[:, None] * s[None, :] / BS
    wr = np.einsum("ijs,ks->ijk", W, np.cos(ang))
    wi = np.einsum("ijs,ks->ijk", W, np.sin(ang))
    M = np.zeros((NK, 128, 128), np.float32)

    def colperm(Wblk):
        # [i, j] -> [row j-perm, col i-perm]: cols 32*(2*par) + a ordering
        # returns [64 j, 64 i-col] for one c2 quadrant pair handled below
        return Wblk

    # row order r: 0..31 even-j Re, 32..63 even-j Im, 64..95 odd-j Re, 96..127 odd-j Im
    # col order m: 32*(2*par + c2) + a  for i = 2a+par, c2 in {Re:0, Im:1}
    icol = np.empty(64, np.int64)  # icol[i-block] base col group by parity
    for i in range(64):
        a, par = divmod(i, 2)
        icol[i] = 64 * par + a  # Re col for block i; Im col = +32
    for kk in range(NK):
        if kk == 0:
            WrE, WiE = wr[:, :, 0], None   # bins 0 / 32 packed
            W32 = wr[:, :, 32]
        Wr, Wi = wr[:, :, kk], wi[:, :, kk]
        for par_j in range(2):
            jrows = np.arange(32) * 2 + par_j       # j block index
            rre = 64 * par_j + np.arange(32)        # row for (a_j, par_j, Re)
            rim = rre + 32
            for i in range(64):
                cre = icol[i]
                cim = cre + 32
                if kk == 0:
                    M[0, rre, cre] = wr[i, jrows, 0]
                    M[0, rim, cim] = W32[i, jrows]
                else:
                    M[kk, rre, cre] = Wr[i, jrows]
                    M[kk, rim, cre] = -Wi[i, jrows]
                    M[kk, rre, cim] = Wi[i, jrows]
                    M[kk, rim, cim] = Wr[i, jrows]
    # device layout: [row r, tile k, col m]
    return np.ascontiguousarray(M.transpose(1, 0, 2))


def prep_x(x):
    """[B, 4096] -> per-core [NH, 128, 32, BW]; p = par*64+t, j = 2a+par."""
    xr = x.reshape(N_CORES, NH, BW, 32, 2, 64)  # [c, h, w, a, par, t]
    xp = np.ascontiguousarray(xr.transpose(0, 1, 4, 5, 3, 2))  # [c,h,par,t,a,w]
    return xp.reshape(N_CORES, NH, 128, 32, BW)


def post_y(ys):
    """per-core [NH, 128, BW, NA] -> [B, 4096]; p = par*64 + tau, i = 2a+par."""
    y = np.stack(ys)  # [c, NH, 128, BW, NA]
    y = y.reshape(N_CORES, NH, 2, 64, BW, NA)  # [c,h,par,tau,w,a]
    y = y.transpose(0, 1, 4, 5, 2, 3)  # [c,h,w,a,par,tau]
    return np.ascontiguousarray(y).reshape(B, OUT)


# ---------------- device kernel ----------------

def evac_engine(idx):
    # split PSUM evacuation between DVE and ACT (DVE also does transposes)
    return "vector" if idx % 3 == 2 else "scalar"


def _evac(nc, eng, dst, src):
    if eng == "vector":
        return nc.vector.tensor_copy(dst, src)
    return nc.scalar.copy(dst, src)


def build_nc(reps=1):
    """v3: shuffles via DVE StreamTranspose (32x32 block transposes).

    Spectral tiles live in (w-major, comp-inner) free layout:
      s_sb [128, BW, NA]: f = w*32 + a   (comp rows, per-jpair chunks)
      s2   [128, BW, NK]: f = w*32 + k   (bin-major rows after transpose)
      o_sb [128, BW, NK]: f = w*32 + k
      v    [128, BW, NA]: f = w*32 + a
    DVE block-transpose swaps (a<->row-within-quarter) per 32x32 block.
    """
    nc = bacc.Bacc("TRN2", target_bir_lowering=False, debug=False,
                   num_devices=N_CORES, dynamic_dma_scratch_size=8192)
    x_in = nc.dram_tensor("x", [NH, 128, NA, BW], MM_DT, kind="ExternalInput")
    fmat = nc.dram_tensor("fmat", [128, 128], MM_DT, kind="ExternalInput")
    gmat = nc.dram_tensor("gmat", [128, 128], F32, kind="ExternalInput")
    wmat = nc.dram_tensor("wmat", [128, NK, 128], F32, kind="ExternalInput")
    y_out = nc.dram_tensor("y", [NH, 128, BW, NA], F32, kind="ExternalOutput")

    ASPL = NA // IN_SPLIT

    with tile.TileContext(nc) as tc:
        with (
            tc.tile_pool(name="consts", bufs=1) as cpool,
            tc.tile_pool(name="p1", bufs=2) as p1,   # xw / o_sb
            tc.tile_pool(name="p2", bufs=2) as p2,   # s_sb / v
            tc.tile_pool(name="p3", bufs=2) as p3,   # s2 / y
            tc.tile_pool(name="fps", bufs=3, space="PSUM") as fps,
            tc.tile_pool(name="mps", bufs=2, space="PSUM") as mps,
            tc.tile_pool(name="ips", bufs=3, space="PSUM") as ips,
        ):
            f_sb = cpool.tile([128, 128], MM_DT)
            g_sb = cpool.tile([128, 128], F32)
            w_sb = cpool.tile([128, NK, 128], F32)
            nc.sync.dma_start(f_sb[:], fmat[:])
            nc.sync.dma_start(g_sb[:], gmat[:])
            nc.sync.dma_start(w_sb[:], wmat[:])

            for _ in range(reps):
                for h in range(NH):
                    xw = p1.tile([128, NA, BW], MM_DT, tag="a")
                    for s in range(IN_SPLIT):
                        getattr(nc, IO_ENGINES[0]).dma_start(
                            xw[:, s * ASPL:(s + 1) * ASPL, :],
                            x_in[h, :, s * ASPL:(s + 1) * ASPL, :])
                    last = xw

                    # FWD: out columns ordered (w, a2) to match s_sb layout
                    s_sb = p2.tile([128, BW, NA], F32, tag="b")
                    for a in [] if LEVEL < 1 else range(0, NA, 2):
                        ps = fps.tile([128, BW, 2], F32, tag="fps")
                        rhs = xw[:, a:a + 2, :].rearrange("p a w -> p w a")
                        nc.tensor.matmul(ps[:], f_sb[:], rhs)
                        _evac(nc, evac_engine(a // 2), s_sb[:, :, a:a + 2],
                              ps[:])
                    if LEVEL >= 1:
                        last = s_sb

                    s2 = p3.tile([128, BW, NK], F32, tag="c")
                    if LEVEL >= 2:
                        nc.vector.transpose(s2[:], s_sb[:])
                        last = s2

                    o_sb = p1.tile([128, BW, NK], F32, tag="a")
                    for k in [] if LEVEL < 3 else range(NK):
                        ps = mps.tile([128, BW], F32, tag="mps")
                        nc.tensor.matmul(ps[:], w_sb[:, k, :], s2[:, :, k])
                        _evac(nc, evac_engine(k + 1), o_sb[:, :, k], ps[:])
                    if LEVEL >= 3:
                        last = o_sb

                    v_sb = p2.tile([128, BW, NA], F32, tag="b")
                    if LEVEL >= 4:
                        nc.vector.transpose(v_sb[:], o_sb[:])
                        last = v_sb

                    y_sb = p3.tile([128, BW, NA], F32, tag="c")
                    for a in [] if LEVEL < 5 else range(0, NA, 2):
                        ps = ips.tile([128, BW, 2], F32, tag="ips")
                        nc.tensor.matmul(ps[:], g_sb[:], v_sb[:, :, a:a + 2])
                        _evac(nc, evac_engine(a // 2 + 2),
                              y_sb[:, :, a:a + 2], ps[:])

                    out_src = y_sb if LEVEL >= 5 else last
                    WSPL = BW // IN_SPLIT
                    for s in range(IN_SPLIT):
                        if LEVEL >= 5:
                            getattr(nc, IO_ENGINES[1]).dma_start(
                                y_out[h, :, s * WSPL:(s + 1) * WSPL, :],
                                y_sb[:, s * WSPL:(s + 1) * WSPL, :])
                        else:
                            getattr(nc, IO_ENGINES[1]).dma_start(
                                y_out[h].rearrange("p w c -> p (w c)")[
                                    :, s * (NA * BW // IN_SPLIT):
                                    (s + 1) * (NA * BW // IN_SPLIT)],
                                out_src[:].rearrange(
                                    "p w c -> p (w c)" if last is not xw
                                    else "p c w -> p (c w)")[
                                    :, s * (NA * BW // IN_SPLIT):
                                    (s + 1) * (NA * BW // IN_SPLIT)])

    nc.compile()
    return nc


_NC_CACHE = {}


def run(x, W, reps=1):
    if reps not in _NC_CACHE:
        _NC_CACHE[reps] = build_nc(reps)
    nc = _NC_CACHE[reps]
    fmat = make_fmat()
    gmat = make_gmat()
    wmat = make_wmats(np.asarray(W, np.float32))
    xp = prep_x(np.ascontiguousarray(np.asarray(x, np.float32)))
    in_maps = [
        {"x": xp[c], "fmat": fmat, "gmat": gmat, "wmat": wmat}
        for c in range(N_CORES)
    ]
    res = run_bass_kernel_spmd(nc, in_maps, list(range(N_CORES)))
    return post_y([res.results[c]["y"] for c in range(N_CORES)])




_NC = None


def kernel(x, W):
    global _NC
    if _NC is None:
        _NC = build_nc(reps=1)
    fmat = make_fmat()
    gmat = make_gmat()
    wmat = make_wmats(np.asarray(W, np.float32))
    xp = prep_x(np.ascontiguousarray(np.asarray(x, np.float32)))
    in_maps = [
        {"x": xp[c], "fmat": fmat, "gmat": gmat, "wmat": wmat}
        for c in range(N_CORES)
    ]
    res = run_bass_kernel_spmd(nc=_NC, in_maps=in_maps,
                               core_ids=list(range(N_CORES)))
    return post_y([res.results[c]["y"] for c in range(N_CORES)])



# revision 2
# speedup vs baseline: 3.9838x; 3.9838x over previous
"""Trainium2 kernel: block-circulant FFT linear layer (bf16, f32-pair pivots).

Over v3 (u64 DVE elements are ISA-illegal, so pairs stay the pivot unit):
  - pipeline order fixed: T2(r) is emitted before fwd(r+1), so inv(r)
    never waits behind fwd(r+1)'s DVE work
  - s2 layout [b, k, s, w2] (wp = 2b+s): T1 out is only 2-strided and
    the mid stage reads 8B contiguous runs (balance of v2/v3 extremes)
  - evac split ACT 32 / DVE 16; T1(r+1) emitted after inv(r) so the
    DVE queue serves inv's evacuations when the PE needs them

kernel(x, W): x [4096, 4096] f32, W [64, 64, 64] f32 -> [4096, 4096] f32.
"""
import numpy as np
import ml_dtypes
import concourse.bass as bass
import concourse.bacc as bacc
import concourse.mybir as mybir
import concourse.tile as tile
from concourse.bass_utils import run_bass_kernel_spmd

N_CORES = 8
B, IN, OUT, BS = 4096, 4096, 4096, 64
BC = B // N_CORES            # 512 batch rows per core
WP = BC // 2                 # 256 w-pairs per core
NA = 32
NK = 32
WC = 64                      # w-pairs per transpose/DMA chunk
NWQ = WP // WC               # 4 chunks
BF16 = mybir.dt.bfloat16
F32 = mybir.dt.float32


# ---------------- host-side constant matrices ----------------

def make_fmat():
    t = np.arange(BS)[:, None]
    c = np.arange(BS)[None, :]
    k = np.where(c <= 32, c, c - 32)
    ang = 2 * np.pi * k * t / BS
    F = np.where(c <= 32, np.cos(ang), np.sin(ang))
    bd = np.zeros((128, 128), np.float32)
    bd[:64, :64] = F
    bd[64:, 64:] = F
    return bd.astype(ml_dtypes.bfloat16)


def make_gmat():
    tau = np.arange(BS)[None, :]
    c = np.arange(BS)[:, None]
    k = np.where(c <= 32, c, c - 32)
    ang = 2 * np.pi * k * tau / BS
    base = np.where(c <= 32, np.cos(ang), np.sin(ang))
    scale = np.where((c % 32) == 0, 1.0 / BS, 2.0 / BS)
    G = base * scale
    bd = np.zeros((128, 128), np.float32)
    bd[:64, :64] = G
    bd[64:, 64:] = G
    return bd.astype(ml_dtypes.bfloat16)


def make_wmats(W):
    W = np.asarray(W, np.float32)
    s = np.arange(BS)
    k = np.arange(33)
    ang = 2 * np.pi * k[:, None] * s[None, :] / BS
    wr = np.einsum("ijs,ks->ijk", W, np.cos(ang))
    wi = np.einsum("ijs,ks->ijk", W, np.sin(ang))
    M = np.zeros((NK, 128, 128), np.float32)
    icol = np.empty(64, np.int64)
    for i in range(64):
        a, par = divmod(i, 2)
        icol[i] = 64 * par + a
    for kk in range(NK):
        if kk == 0:
            W32 = wr[:, :, 32]
        Wr, Wi = wr[:, :, kk], wi[:, :, kk]
        for par_j in range(2):
            jrows = np.arange(32) * 2 + par_j
            rre = 64 * par_j + np.arange(32)
            rim = rre + 32
            for i in range(64):
                cre = icol[i]
                cim = cre + 32
                if kk == 0:
                    M[0, rre, cre] = wr[i, jrows, 0]
                    M[0, rim, cim] = W32[i, jrows]
                else:
                    M[kk, rre, cre] = Wr[i, jrows]
                    M[kk, rim, cre] = -Wi[i, jrows]
                    M[kk, rre, cim] = Wi[i, jrows]
                    M[kk, rim, cim] = Wr[i, jrows]
    return np.ascontiguousarray(M.transpose(1, 0, 2)).astype(ml_dtypes.bfloat16)


def prep_x(x):
    """[B, 4096] f32 -> per-core [128, WP, NA, 2] bf16.

    partition p = par*64 + t (j = 2a+par); free = (w-pair, a, w-parity)."""
    xr = np.asarray(x, np.float32).reshape(N_CORES, WP, 2, NA, 2, 64)
    xp = xr.transpose(0, 4, 5, 1, 3, 2)      # [c, par, t, wp, a, w2]
    return np.ascontiguousarray(xp).reshape(
        N_CORES, 128, WP, NA, 2).astype(ml_dtypes.bfloat16)


def post_y(ys):
    """per-core [128, WP, NA, 2] bf16 -> [B, 4096] f32; p = par*64 + tau,
    i = 2a+par."""
    y = np.stack(ys).astype(np.float32)      # [c, 128, WP, NA, 2]
    y = y.reshape(N_CORES, 2, 64, WP, NA, 2)  # [c, par, tau, wp, a, w2]
    y = y.transpose(0, 3, 5, 4, 1, 2)        # [c, wp, w2, a, par, tau]
    return np.ascontiguousarray(y).reshape(B, OUT)


# ---------------- device kernel ----------------

def build_nc(reps=1):
    nc = bacc.Bacc("TRN2", target_bir_lowering=False, debug=False,
                   num_devices=N_CORES, dynamic_dma_scratch_size=8192)
    x_in = nc.dram_tensor("x", [128, WP, NA, 2], BF16, kind="ExternalInput")
    fmat = nc.dram_tensor("fmat", [128, 128], BF16, kind="ExternalInput")
    gmat = nc.dram_tensor("gmat", [128, 128], BF16, kind="ExternalInput")
    wmat = nc.dram_tensor("wmat", [128, NK, 128], BF16, kind="ExternalInput")
    y_out = nc.dram_tensor("y", [128, WP, NA, 2], BF16, kind="ExternalOutput")

    with tile.TileContext(nc) as tc:
        with (
            tc.tile_pool(name="consts", bufs=1) as cpool,
            tc.tile_pool(name="px", bufs=2) as px,
            tc.tile_pool(name="pt", bufs=1) as pt,
            tc.tile_pool(name="ps", bufs=4, space="PSUM") as psp,
        ):
            f_sb = cpool.tile([128, 128], BF16)
            g_sb = cpool.tile([128, 128], BF16)
            w_sb = cpool.tile([128, NK, 128], BF16)
            nc.sync.dma_start(f_sb[:], fmat[:])
            nc.sync.dma_start(g_sb[:], gmat[:])
            nc.sync.dma_start(w_sb[:], wmat[:])

            def alloc_x(r):
                xh = []
                for h in range(2):
                    xt = px.tile([128, WP // 2, NA, 2], BF16, tag="x",
                                 name=f"xt{h}")
                    nc.sync.dma_start(
                        xt[:], x_in[:, h * (WP // 2):(h + 1) * (WP // 2)])
                    xh.append(xt)
                return xh

            def emit_fwd(xh):
                s_sb = pt.tile([128, WP, NA, 2], BF16, tag="s", name="s_sb")
                # s2: wp = 2*b + s; [b, k, s, w2] so T1-out is 2-strided in
                # k and the mid stage reads 8B-contiguous (s, w2) runs
                s2 = pt.tile([128, WP // 2, NK, 2, 2], BF16, tag="s2",
                             name="s2")
                for wq in range(NWQ):
                    h, hw = wq // 2, (wq % 2) * WC
                    ws = slice(wq * WC, (wq + 1) * WC)
                    for g in range(4):
                        ps = psp.tile([128, 2, WC, 4, 2], F32, tag="ps",
                                      name="psf")
                        for j in range(2):
                            a0 = g * 8 + j * 4
                            nc.tensor.matmul(
                                ps[:, j], f_sb[:],
                                xh[h][:, hw:hw + WC, a0:a0 + 4, :])
                        dst = s_sb[:, ws, g * 8:(g + 1) * 8, :].rearrange(
                            "p w (j a) c -> p j w a c", j=2)
                        if g == 1:
                            nc.vector.tensor_copy(dst, ps[:])
                        else:
                            nc.scalar.copy(dst, ps[:])
                return s_sb, s2

            def emit_t1(s_sb, s2):
                for wq in range(NWQ):
                    ws = slice(wq * WC, (wq + 1) * WC)
                    bs = slice(wq * (WC // 2), (wq + 1) * (WC // 2))
                    nc.vector.transpose(
                        s2[:, bs].bitcast(F32).rearrange(
                            "p b k s x -> p b s k x"),
                        s_sb[:, ws].bitcast(F32))

            def emit_mid(s2):
                o_sb = pt.tile([128, WP, NK, 2], BF16, tag="o", name="o_sb")
                for kp in range(NK // 2):
                    ps = psp.tile([128, 2, WP // 2, 2, 2], F32, tag="ps",
                                  name="psm")
                    for j in range(2):
                        k = kp * 2 + j
                        nc.tensor.matmul(
                            ps[:, j], w_sb[:, k, :], s2[:, :, k, :, :])
                    dst = o_sb[:, :, kp * 2:kp * 2 + 2, :].rearrange(
                        "p (b s) k c -> p k b s c", s=2)
                    if kp % 4 == 1:
                        nc.vector.tensor_copy(dst, ps[:])
                    else:
                        nc.scalar.copy(dst, ps[:])
                return o_sb

            def emit_t2(o_sb):
                v_sb = pt.tile([128, WP, NA, 2], BF16, tag="v", name="v_sb")
                for wq in range(NWQ):
                    ws = slice(wq * WC, (wq + 1) * WC)
                    nc.vector.transpose(
                        v_sb[:, ws].bitcast(F32), o_sb[:, ws].bitcast(F32))
                return v_sb

            def emit_inv(v_sb, pending_out):
                y_sb = pt.tile([128, WP, NA, 2], BF16, tag="y", name="y_sb")
                n = 0
                for wq in range(NWQ):
                    ws = slice(wq * WC, (wq + 1) * WC)
                    for g in range(4):
                        ps = psp.tile([128, 2, WC, 4, 2], F32, tag="ps",
                                      name="psi")
                        for j in range(2):
                            a0 = g * 8 + j * 4
                            nc.tensor.matmul(
                                ps[:, j], g_sb[:],
                                v_sb[:, ws, a0:a0 + 4, :])
                        dst = y_sb[:, ws, g * 8:(g + 1) * 8, :].rearrange(
                            "p w (j a) c -> p j w a c", j=2)
                        if n % 2 == 0:
                            nc.vector.tensor_copy(dst, ps[:])
                        else:
                            nc.scalar.copy(dst, ps[:])
                        n += 1
                    pending_out.append((y_out[:, ws], y_sb[:, ws]))

            # software pipeline: PE order [mid(r), fwd(r+1), inv(r)];
            # DVE order [mid-casts(r), T2(r), fwd-casts+T1(r+1), inv-casts(r)]
            pending_out = []
            xh = alloc_x(0)
            s_sb, s2 = emit_fwd(xh)
            emit_t1(s_sb, s2)
            for r in range(reps):
                if r + 1 < reps:
                    xh = alloc_x(r + 1)
                o_sb = emit_mid(s2)
                v_sb = emit_t2(o_sb)
                if r + 1 < reps:
                    s_sb, s2 = emit_fwd(xh)
                emit_inv(v_sb, pending_out)
                if r + 1 < reps:
                    emit_t1(s_sb, s2)
                for dst, src in pending_out:
                    nc.sync.dma_start(dst, src)
                pending_out = []

    nc.compile()
    return nc


_NC_CACHE = {}


def _in_maps(x, W):
    fmat = make_fmat()
    gmat = make_gmat()
    wmat = make_wmats(W)
    xp = prep_x(x)
    return [
        {"x": xp[c], "fmat": fmat, "gmat": gmat, "wmat": wmat}
        for c in range(N_CORES)
    ]


def run(x, W, reps=1):
    if reps not in _NC_CACHE:
        _NC_CACHE[reps] = build_nc(reps)
    nc = _NC_CACHE[reps]
    res = run_bass_kernel_spmd(nc, _in_maps(x, W), list(range(N_CORES)))
    return post_y([res.results[c]["y"] for c in range(N_CORES)])


def kernel(x, W):
    if 1 not in _NC_CACHE:
        _NC_CACHE[1] = build_nc(reps=1)
    res = run_bass_kernel_spmd(nc=_NC_CACHE[1], in_maps=_in_maps(x, W),
                               core_ids=list(range(N_CORES)))
    return post_y([res.results[c]["y"] for c in range(N_CORES)])


# revision 3
# speedup vs baseline: 4.4379x; 1.1140x over previous
"""Trainium2 kernel: block-circulant FFT linear layer (bf16, pair-packed pivots).

Over v3 (u64 DVE elements are ISA-illegal, so pairs stay the pivot unit):
  - pipeline order fixed: T2(r) is emitted before fwd(r+1), so inv(r)
    never waits behind fwd(r+1)'s DVE work
  - s2 layout [b, k, s, w2] (wp = 2b+s): T1 out is only 2-strided and
    the mid stage reads 8B contiguous runs (balance of v2/v3 extremes)
  - evac split ACT 32 / DVE 16; T1(r+1) emitted after inv(r) so the
    DVE queue serves inv's evacuations when the PE needs them

kernel(x, W): x [4096, 4096] f32, W [64, 64, 64] f32 -> [4096, 4096] f32.
"""
import numpy as np
import ml_dtypes
import concourse.bass as bass
import concourse.bacc as bacc
import concourse.mybir as mybir
import concourse.tile as tile
from concourse.bass_utils import run_bass_kernel_spmd

N_CORES = 8
B, IN, OUT, BS = 4096, 4096, 4096, 64
BC = B // N_CORES            # 512 batch rows per core
WP = BC // 2                 # 256 w-pairs per core
NA = 32
NK = 32
WC = 64                      # w-pairs per transpose/DMA chunk
NWQ = WP // WC               # 4 chunks
BF16 = mybir.dt.bfloat16
F32 = mybir.dt.float32


# ---------------- host-side constant matrices ----------------

def make_fmat():
    t = np.arange(BS)[:, None]
    c = np.arange(BS)[None, :]
    k = np.where(c <= 32, c, c - 32)
    ang = 2 * np.pi * k * t / BS
    F = np.where(c <= 32, np.cos(ang), np.sin(ang))
    bd = np.zeros((128, 128), np.float32)
    bd[:64, :64] = F
    bd[64:, 64:] = F
    return bd.astype(ml_dtypes.bfloat16)


def make_gmat():
    tau = np.arange(BS)[None, :]
    c = np.arange(BS)[:, None]
    k = np.where(c <= 32, c, c - 32)
    ang = 2 * np.pi * k * tau / BS
    base = np.where(c <= 32, np.cos(ang), np.sin(ang))
    scale = np.where((c % 32) == 0, 1.0 / BS, 2.0 / BS)
    G = base * scale
    bd = np.zeros((128, 128), np.float32)
    bd[:64, :64] = G
    bd[64:, 64:] = G
    return bd.astype(ml_dtypes.bfloat16)


def make_wmats(W):
    W = np.asarray(W, np.float32)
    s = np.arange(BS)
    k = np.arange(33)
    ang = 2 * np.pi * k[:, None] * s[None, :] / BS
    wr = np.einsum("ijs,ks->ijk", W, np.cos(ang))
    wi = np.einsum("ijs,ks->ijk", W, np.sin(ang))
    M = np.zeros((NK, 128, 128), np.float32)
    icol = np.empty(64, np.int64)
    for i in range(64):
        a, par = divmod(i, 2)
        icol[i] = 64 * par + a
    for kk in range(NK):
        if kk == 0:
            W32 = wr[:, :, 32]
        Wr, Wi = wr[:, :, kk], wi[:, :, kk]
        for par_j in range(2):
            jrows = np.arange(32) * 2 + par_j
            rre = 64 * par_j + np.arange(32)
            rim = rre + 32
            for i in range(64):
                cre = icol[i]
                cim = cre + 32
                if kk == 0:
                    M[0, rre, cre] = wr[i, jrows, 0]
                    M[0, rim, cim] = W32[i, jrows]
                else:
                    M[kk, rre, cre] = Wr[i, jrows]
                    M[kk, rim, cre] = -Wi[i, jrows]
                    M[kk, rre, cim] = Wi[i, jrows]
                    M[kk, rim, cim] = Wr[i, jrows]
    return np.ascontiguousarray(M.transpose(1, 0, 2)).astype(ml_dtypes.bfloat16)


def prep_x(x):
    """[B, 4096] f32 -> per-core [128, WP, NA, 2] bf16.

    partition p = par*64 + t (j = 2a+par); free = (w-pair, a, w-parity)."""
    xr = np.asarray(x, np.float32).reshape(N_CORES, WP, 2, NA, 2, 64)
    xp = xr.transpose(0, 4, 5, 1, 3, 2)      # [c, par, t, wp, a, w2]
    return np.ascontiguousarray(xp).reshape(
        N_CORES, 128, WP, NA, 2).astype(ml_dtypes.bfloat16)


def post_y(ys):
    """per-core [128, WP, NA, 2] bf16 -> [B, 4096] f32; p = par*64 + tau,
    i = 2a+par."""
    y = np.stack(ys).astype(np.float32)      # [c, 128, WP, NA, 2]
    y = y.reshape(N_CORES, 2, 64, WP, NA, 2)  # [c, par, tau, wp, a, w2]
    y = y.transpose(0, 3, 5, 4, 1, 2)        # [c, wp, w2, a, par, tau]
    return np.ascontiguousarray(y).reshape(B, OUT)


# ---------------- device kernel ----------------

def build_nc(reps=1):
    nc = bacc.Bacc("TRN2", target_bir_lowering=False, debug=False,
                   num_devices=N_CORES, dynamic_dma_scratch_size=8192)
    x_in = nc.dram_tensor("x", [128, WP, NA, 2], BF16, kind="ExternalInput")
    fmat = nc.dram_tensor("fmat", [128, 128], BF16, kind="ExternalInput")
    gmat = nc.dram_tensor("gmat", [128, 128], BF16, kind="ExternalInput")
    wmat = nc.dram_tensor("wmat", [128, NK, 128], BF16, kind="ExternalInput")
    y_out = nc.dram_tensor("y", [128, WP, NA, 2], BF16, kind="ExternalOutput")

    with tile.TileContext(nc) as tc:
        with (
            tc.tile_pool(name="consts", bufs=1) as cpool,
            tc.tile_pool(name="px", bufs=2) as px,
            tc.tile_pool(name="pt", bufs=1) as pt,
            tc.tile_pool(name="ps", bufs=4, space="PSUM") as psp,
        ):
            f_sb = cpool.tile([128, 128], BF16)
            g_sb = cpool.tile([128, 128], BF16)
            w_sb = cpool.tile([128, NK, 128], BF16)
            nc.sync.dma_start(f_sb[:], fmat[:])
            nc.sync.dma_start(g_sb[:], gmat[:])
            nc.sync.dma_start(w_sb[:], wmat[:])

            def alloc_x(r):
                xh = []
                for h in range(2):
                    xt = px.tile([128, WP // 2, NA, 2], BF16, tag="x",
                                 name=f"xt{h}")
                    nc.sync.dma_start(
                        xt[:], x_in[:, h * (WP // 2):(h + 1) * (WP // 2)])
                    xh.append(xt)
                return xh

            def emit_fwd(xh):
                s_sb = pt.tile([128, WP, NA, 2], BF16, tag="s", name="s_sb")
                # s2: wp = 2*b + s; [b, k, s, w2] so T1-out is 2-strided in
                # k and the mid stage reads 8B-contiguous (s, w2) runs
                s2 = pt.tile([128, WP // 2, NK, 2, 2], BF16, tag="s2",
                             name="s2")
                for wq in range(NWQ):
                    h, hw = wq // 2, (wq % 2) * WC
                    ws = slice(wq * WC, (wq + 1) * WC)
                    for g in range(4):
                        ps = psp.tile([128, 2, WC, 4, 2], F32, tag="ps",
                                      name="psf")
                        for j in range(2):
                            a0 = g * 8 + j * 4
                            nc.tensor.matmul(
                                ps[:, j], f_sb[:],
                                xh[h][:, hw:hw + WC, a0:a0 + 4, :])
                        dst = s_sb[:, ws, g * 8:(g + 1) * 8, :].rearrange(
                            "p w (j a) c -> p j w a c", j=2)
                        if g == 1:
                            nc.vector.tensor_copy(dst, ps[:])
                        else:
                            nc.scalar.copy(dst, ps[:])
                return s_sb, s2

            def emit_t1(s_sb, s2):
                for wq in range(NWQ):
                    ws = slice(wq * WC, (wq + 1) * WC)
                    bs = slice(wq * (WC // 2), (wq + 1) * (WC // 2))
                    nc.vector.transpose(
                        s2[:, bs].bitcast(F32).rearrange(
                            "p b k s x -> p b s k x"),
                        s_sb[:, ws].bitcast(F32))

            def emit_mid(s2):
                o_sb = pt.tile([128, WP, NK, 2], BF16, tag="o", name="o_sb")
                NB = WP // 2      # 128 b-lines
                NSPLIT = 4        # first 4 k-pairs run as b-halves so the
                                  # PE can start before T1's last chunks land

                def emit_evac(kp, ps):
                    dst = o_sb[:, :, kp * 2:kp * 2 + 2, :].rearrange(
                        "p (b s) k c -> p k b s c", s=2)
                    if kp % 4 == 1:
                        nc.vector.tensor_copy(dst, ps[:])
                    else:
                        nc.scalar.copy(dst, ps[:])

                pstiles = []
                for kp in range(NSPLIT):
                    ps = psp.tile([128, 2, NB, 2, 2], F32, tag="ps",
                                  name="psm")
                    pstiles.append(ps)
                    for j in range(2):
                        k = kp * 2 + j
                        nc.tensor.matmul(
                            ps[:, j, :NB // 2], w_sb[:, k, :],
                            s2[:, :NB // 2, k, :, :])
                for kp in range(NSPLIT):
                    ps = pstiles[kp]
                    for j in range(2):
                        k = kp * 2 + j
                        nc.tensor.matmul(
                            ps[:, j, NB // 2:], w_sb[:, k, :],
                            s2[:, NB // 2:, k, :, :])
                    emit_evac(kp, ps)
                for kp in range(NSPLIT, NK // 2):
                    ps = psp.tile([128, 2, NB, 2, 2], F32, tag="ps",
                                  name="psm")
                    for j in range(2):
                        k = kp * 2 + j
                        nc.tensor.matmul(
                            ps[:, j], w_sb[:, k, :], s2[:, :, k, :, :])
                    emit_evac(kp, ps)
                return o_sb

            def emit_t2(o_sb):
                v_sb = pt.tile([128, WP, NA, 2], BF16, tag="v", name="v_sb")
                for wq in range(NWQ):
                    ws = slice(wq * WC, (wq + 1) * WC)
                    nc.vector.transpose(
                        v_sb[:, ws].bitcast(F32), o_sb[:, ws].bitcast(F32))
                return v_sb

            def emit_inv(v_sb, pending_out):
                y_sb = pt.tile([128, WP, NA, 2], BF16, tag="y", name="y_sb")
                n = 0
                for wq in range(NWQ):
                    ws = slice(wq * WC, (wq + 1) * WC)
                    for g in range(4):
                        ps = psp.tile([128, 2, WC, 4, 2], F32, tag="ps",
                                      name="psi")
                        for j in range(2):
                            a0 = g * 8 + j * 4
                            nc.tensor.matmul(
                                ps[:, j], g_sb[:],
                                v_sb[:, ws, a0:a0 + 4, :])
                        dst = y_sb[:, ws, g * 8:(g + 1) * 8, :].rearrange(
                            "p w (j a) c -> p j w a c", j=2)
                        if n % 2 == 0:
                            nc.vector.tensor_copy(dst, ps[:])
                        else:
                            nc.scalar.copy(dst, ps[:])
                        n += 1
                    pending_out.append((y_out[:, ws], y_sb[:, ws]))

            # software pipeline: PE order [mid(r), fwd(r+1), inv(r)];
            # DVE order [mid-casts(r), T2(r), fwd-casts+T1(r+1), inv-casts(r)]
            pending_out = []
            xh = alloc_x(0)
            s_sb, s2 = emit_fwd(xh)
            emit_t1(s_sb, s2)
            for r in range(reps):
                if r + 1 < reps:
                    xh = alloc_x(r + 1)
                o_sb = emit_mid(s2)
                v_sb = emit_t2(o_sb)
                if r + 1 < reps:
                    s_sb, s2 = emit_fwd(xh)
                emit_inv(v_sb, pending_out)
                if r + 1 < reps:
                    emit_t1(s_sb, s2)
                for dst, src in pending_out:
                    nc.sync.dma_start(dst, src)
                pending_out = []

    nc.compile()
    return nc


_NC_CACHE = {}


def _in_maps(x, W):
    fmat = make_fmat()
    gmat = make_gmat()
    wmat = make_wmats(W)
    xp = prep_x(x)
    return [
        {"x": xp[c], "fmat": fmat, "gmat": gmat, "wmat": wmat}
        for c in range(N_CORES)
    ]


def run(x, W, reps=1):
    if reps not in _NC_CACHE:
        _NC_CACHE[reps] = build_nc(reps)
    nc = _NC_CACHE[reps]
    res = run_bass_kernel_spmd(nc, _in_maps(x, W), list(range(N_CORES)))
    return post_y([res.results[c]["y"] for c in range(N_CORES)])


def kernel(x, W):
    if 1 not in _NC_CACHE:
        _NC_CACHE[1] = build_nc(reps=1)
    res = run_bass_kernel_spmd(nc=_NC_CACHE[1], in_maps=_in_maps(x, W),
                               core_ids=list(range(N_CORES)))
    return post_y([res.results[c]["y"] for c in range(N_CORES)])


# revision 4
# speedup vs baseline: 5.0097x; 1.1289x over previous
"""Trainium2 kernel: block-circulant FFT linear layer (bf16, pair-packed pivots,
split ACT/DVE PSUM rings).

Over v3 (u64 DVE elements are ISA-illegal, so pairs stay the pivot unit):
  - pipeline order fixed: T2(r) is emitted before fwd(r+1), so inv(r)
    never waits behind fwd(r+1)'s DVE work
  - s2 layout [b, k, s, w2] (wp = 2b+s): T1 out is only 2-strided and
    the mid stage reads 8B contiguous runs (balance of v2/v3 extremes)
  - evac split ACT 32 / DVE 16; T1(r+1) emitted after inv(r) so the
    DVE queue serves inv's evacuations when the PE needs them

kernel(x, W): x [4096, 4096] f32, W [64, 64, 64] f32 -> [4096, 4096] f32.
"""
import numpy as np
import ml_dtypes
import concourse.bass as bass
import concourse.bacc as bacc
import concourse.mybir as mybir
import concourse.tile as tile
from concourse.bass_utils import run_bass_kernel_spmd

N_CORES = 8
B, IN, OUT, BS = 4096, 4096, 4096, 64
BC = B // N_CORES            # 512 batch rows per core
WP = BC // 2                 # 256 w-pairs per core
NA = 32
NK = 32
WC = 64                      # w-pairs per transpose/DMA chunk
NWQ = WP // WC               # 4 chunks
BF16 = mybir.dt.bfloat16
F32 = mybir.dt.float32


# ---------------- host-side constant matrices ----------------

def make_fmat():
    t = np.arange(BS)[:, None]
    c = np.arange(BS)[None, :]
    k = np.where(c <= 32, c, c - 32)
    ang = 2 * np.pi * k * t / BS
    F = np.where(c <= 32, np.cos(ang), np.sin(ang))
    bd = np.zeros((128, 128), np.float32)
    bd[:64, :64] = F
    bd[64:, 64:] = F
    return bd.astype(ml_dtypes.bfloat16)


def make_gmat():
    tau = np.arange(BS)[None, :]
    c = np.arange(BS)[:, None]
    k = np.where(c <= 32, c, c - 32)
    ang = 2 * np.pi * k * tau / BS
    base = np.where(c <= 32, np.cos(ang), np.sin(ang))
    scale = np.where((c % 32) == 0, 1.0 / BS, 2.0 / BS)
    G = base * scale
    bd = np.zeros((128, 128), np.float32)
    bd[:64, :64] = G
    bd[64:, 64:] = G
    return bd.astype(ml_dtypes.bfloat16)


def make_wmats(W):
    W = np.asarray(W, np.float32)
    s = np.arange(BS)
    k = np.arange(33)
    ang = 2 * np.pi * k[:, None] * s[None, :] / BS
    wr = np.einsum("ijs,ks->ijk", W, np.cos(ang))
    wi = np.einsum("ijs,ks->ijk", W, np.sin(ang))
    M = np.zeros((NK, 128, 128), np.float32)
    icol = np.empty(64, np.int64)
    for i in range(64):
        a, par = divmod(i, 2)
        icol[i] = 64 * par + a
    for kk in range(NK):
        if kk == 0:
            W32 = wr[:, :, 32]
        Wr, Wi = wr[:, :, kk], wi[:, :, kk]
        for par_j in range(2):
            jrows = np.arange(32) * 2 + par_j
            rre = 64 * par_j + np.arange(32)
            rim = rre + 32
            for i in range(64):
                cre = icol[i]
                cim = cre + 32
                if kk == 0:
                    M[0, rre, cre] = wr[i, jrows, 0]
                    M[0, rim, cim] = W32[i, jrows]
                else:
                    M[kk, rre, cre] = Wr[i, jrows]
                    M[kk, rim, cre] = -Wi[i, jrows]
                    M[kk, rre, cim] = Wi[i, jrows]
                    M[kk, rim, cim] = Wr[i, jrows]
    return np.ascontiguousarray(M.transpose(1, 0, 2)).astype(ml_dtypes.bfloat16)


def prep_x(x):
    """[B, 4096] f32 -> per-core [128, WP, NA, 2] bf16.

    partition p = par*64 + t (j = 2a+par); free = (w-pair, a, w-parity)."""
    xr = np.asarray(x, np.float32).reshape(N_CORES, WP, 2, NA, 2, 64)
    xp = xr.transpose(0, 4, 5, 1, 3, 2)      # [c, par, t, wp, a, w2]
    return np.ascontiguousarray(xp).reshape(
        N_CORES, 128, WP, NA, 2).astype(ml_dtypes.bfloat16)


def post_y(ys):
    """per-core [128, WP, NA, 2] bf16 -> [B, 4096] f32; p = par*64 + tau,
    i = 2a+par."""
    y = np.stack(ys).astype(np.float32)      # [c, 128, WP, NA, 2]
    y = y.reshape(N_CORES, 2, 64, WP, NA, 2)  # [c, par, tau, wp, a, w2]
    y = y.transpose(0, 3, 5, 4, 1, 2)        # [c, wp, w2, a, par, tau]
    return np.ascontiguousarray(y).reshape(B, OUT)


# ---------------- device kernel ----------------

def build_nc(reps=1):
    nc = bacc.Bacc("TRN2", target_bir_lowering=False, debug=False,
                   num_devices=N_CORES, dynamic_dma_scratch_size=8192)
    x_in = nc.dram_tensor("x", [128, WP, NA, 2], BF16, kind="ExternalInput")
    fmat = nc.dram_tensor("fmat", [128, 128], BF16, kind="ExternalInput")
    gmat = nc.dram_tensor("gmat", [128, 128], BF16, kind="ExternalInput")
    wmat = nc.dram_tensor("wmat", [128, NK, 128], BF16, kind="ExternalInput")
    y_out = nc.dram_tensor("y", [128, WP, NA, 2], BF16, kind="ExternalOutput")

    with tile.TileContext(nc) as tc:
        with (
            tc.tile_pool(name="consts", bufs=1) as cpool,
            tc.tile_pool(name="px", bufs=2) as px,
            tc.tile_pool(name="pt", bufs=1) as pt,
            tc.tile_pool(name="psa", bufs=2, space="PSUM") as psa,
            tc.tile_pool(name="psd", bufs=2, space="PSUM") as psd,
        ):
            f_sb = cpool.tile([128, 128], BF16)
            g_sb = cpool.tile([128, 128], BF16)
            w_sb = cpool.tile([128, NK, 128], BF16)
            nc.sync.dma_start(f_sb[:], fmat[:])
            nc.sync.dma_start(g_sb[:], gmat[:])
            nc.sync.dma_start(w_sb[:], wmat[:])

            def alloc_x(r):
                xh = []
                for h in range(2):
                    xt = px.tile([128, WP // 2, NA, 2], BF16, tag="x",
                                 name=f"xt{h}")
                    nc.sync.dma_start(
                        xt[:], x_in[:, h * (WP // 2):(h + 1) * (WP // 2)])
                    xh.append(xt)
                return xh

            def emit_fwd(xh):
                s_sb = pt.tile([128, WP, NA, 2], BF16, tag="s", name="s_sb")
                # s2: wp = 2*b + s; [b, k, s, w2] so T1-out is 2-strided in
                # k and the mid stage reads 8B-contiguous (s, w2) runs
                s2 = pt.tile([128, WP // 2, NK, 2, 2], BF16, tag="s2",
                             name="s2")
                for wq in range(NWQ):
                    h, hw = wq // 2, (wq % 2) * WC
                    ws = slice(wq * WC, (wq + 1) * WC)
                    for g in range(4):
                        idx = wq * 4 + g
                        use_d = idx % 3 == 2
                        pool = psd if use_d else psa
                        ps = pool.tile([128, 2, WC, 4, 2], F32, tag="ps",
                                       name="psf")
                        for j in range(2):
                            a0 = g * 8 + j * 4
                            nc.tensor.matmul(
                                ps[:, j], f_sb[:],
                                xh[h][:, hw:hw + WC, a0:a0 + 4, :])
                        dst = s_sb[:, ws, g * 8:(g + 1) * 8, :].rearrange(
                            "p w (j a) c -> p j w a c", j=2)
                        if use_d:
                            nc.vector.tensor_copy(dst, ps[:])
                        else:
                            nc.scalar.copy(dst, ps[:])
                return s_sb, s2

            def emit_t1(s_sb, s2):
                for wq in range(NWQ):
                    ws = slice(wq * WC, (wq + 1) * WC)
                    bs = slice(wq * (WC // 2), (wq + 1) * (WC // 2))
                    nc.vector.transpose(
                        s2[:, bs].bitcast(F32).rearrange(
                            "p b k s x -> p b s k x"),
                        s_sb[:, ws].bitcast(F32))

            def emit_mid(s2):
                o_sb = pt.tile([128, WP, NK, 2], BF16, tag="o", name="o_sb")
                for kp in range(NK // 2):
                    use_d = kp % 3 == 2
                    pool = psd if use_d else psa
                    ps = pool.tile([128, 2, WP // 2, 2, 2], F32, tag="ps",
                                   name="psm")
                    for j in range(2):
                        k = kp * 2 + j
                        nc.tensor.matmul(
                            ps[:, j], w_sb[:, k, :], s2[:, :, k, :, :])
                    dst = o_sb[:, :, kp * 2:kp * 2 + 2, :].rearrange(
                        "p (b s) k c -> p k b s c", s=2)
                    if use_d:
                        nc.vector.tensor_copy(dst, ps[:])
                    else:
                        nc.scalar.copy(dst, ps[:])
                return o_sb

            def emit_t2(o_sb):
                v_sb = pt.tile([128, WP, NA, 2], BF16, tag="v", name="v_sb")
                for wq in range(NWQ):
                    ws = slice(wq * WC, (wq + 1) * WC)
                    nc.vector.transpose(
                        v_sb[:, ws].bitcast(F32), o_sb[:, ws].bitcast(F32))
                return v_sb

            def emit_inv(v_sb, pending_out):
                y_sb = pt.tile([128, WP, NA, 2], BF16, tag="y", name="y_sb")
                n = 0
                for wq in range(NWQ):
                    ws = slice(wq * WC, (wq + 1) * WC)
                    for g in range(4):
                        use_d = n % 3 == 2
                        pool = psd if use_d else psa
                        ps = pool.tile([128, 2, WC, 4, 2], F32, tag="ps",
                                       name="psi")
                        for j in range(2):
                            a0 = g * 8 + j * 4
                            nc.tensor.matmul(
                                ps[:, j], g_sb[:],
                                v_sb[:, ws, a0:a0 + 4, :])
                        dst = y_sb[:, ws, g * 8:(g + 1) * 8, :].rearrange(
                            "p w (j a) c -> p j w a c", j=2)
                        if use_d:
                            nc.vector.tensor_copy(dst, ps[:])
                        else:
                            nc.scalar.copy(dst, ps[:])
                        n += 1
                    pending_out.append((y_out[:, ws], y_sb[:, ws]))

            # software pipeline: PE order [mid(r), fwd(r+1), inv(r)];
            # DVE order [mid-casts(r), T2(r), fwd-casts+T1(r+1), inv-casts(r)]
            pending_out = []
            xh = alloc_x(0)
            s_sb, s2 = emit_fwd(xh)
            emit_t1(s_sb, s2)
            for r in range(reps):
                if r + 1 < reps:
                    xh = alloc_x(r + 1)
                o_sb = emit_mid(s2)
                v_sb = emit_t2(o_sb)
                if r + 1 < reps:
                    s_sb, s2 = emit_fwd(xh)
                emit_inv(v_sb, pending_out)
                if r + 1 < reps:
                    emit_t1(s_sb, s2)
                for dst, src in pending_out:
                    nc.sync.dma_start(dst, src)
                pending_out = []

    nc.compile()
    return nc


_NC_CACHE = {}


def _in_maps(x, W):
    fmat = make_fmat()
    gmat = make_gmat()
    wmat = make_wmats(W)
    xp = prep_x(x)
    return [
        {"x": xp[c], "fmat": fmat, "gmat": gmat, "wmat": wmat}
        for c in range(N_CORES)
    ]


def run(x, W, reps=1):
    if reps not in _NC_CACHE:
        _NC_CACHE[reps] = build_nc(reps)
    nc = _NC_CACHE[reps]
    res = run_bass_kernel_spmd(nc, _in_maps(x, W), list(range(N_CORES)))
    return post_y([res.results[c]["y"] for c in range(N_CORES)])


def kernel(x, W):
    if 1 not in _NC_CACHE:
        _NC_CACHE[1] = build_nc(reps=1)
    res = run_bass_kernel_spmd(nc=_NC_CACHE[1], in_maps=_in_maps(x, W),
                               core_ids=list(range(N_CORES)))
    return post_y([res.results[c]["y"] for c in range(N_CORES)])


# revision 5
# speedup vs baseline: 5.2741x; 1.0528x over previous
"""Trainium2 kernel: block-circulant FFT linear layer (bf16, pair-packed pivots,
ACT triple / DVE single PSUM rings).

Over v3 (u64 DVE elements are ISA-illegal, so pairs stay the pivot unit):
  - pipeline order fixed: T2(r) is emitted before fwd(r+1), so inv(r)
    never waits behind fwd(r+1)'s DVE work
  - s2 layout [b, k, s, w2] (wp = 2b+s): T1 out is only 2-strided and
    the mid stage reads 8B contiguous runs (balance of v2/v3 extremes)
  - evac split ACT 32 / DVE 16; T1(r+1) emitted after inv(r) so the
    DVE queue serves inv's evacuations when the PE needs them

kernel(x, W): x [4096, 4096] f32, W [64, 64, 64] f32 -> [4096, 4096] f32.
"""
import numpy as np
import ml_dtypes
import concourse.bass as bass
import concourse.bacc as bacc
import concourse.mybir as mybir
import concourse.tile as tile
from concourse.bass_utils import run_bass_kernel_spmd

N_CORES = 8
B, IN, OUT, BS = 4096, 4096, 4096, 64
BC = B // N_CORES            # 512 batch rows per core
WP = BC // 2                 # 256 w-pairs per core
NA = 32
NK = 32
WC = 64                      # w-pairs per transpose/DMA chunk
NWQ = WP // WC               # 4 chunks
BF16 = mybir.dt.bfloat16
F32 = mybir.dt.float32


# ---------------- host-side constant matrices ----------------

def make_fmat():
    t = np.arange(BS)[:, None]
    c = np.arange(BS)[None, :]
    k = np.where(c <= 32, c, c - 32)
    ang = 2 * np.pi * k * t / BS
    F = np.where(c <= 32, np.cos(ang), np.sin(ang))
    bd = np.zeros((128, 128), np.float32)
    bd[:64, :64] = F
    bd[64:, 64:] = F
    return bd.astype(ml_dtypes.bfloat16)


def make_gmat():
    tau = np.arange(BS)[None, :]
    c = np.arange(BS)[:, None]
    k = np.where(c <= 32, c, c - 32)
    ang = 2 * np.pi * k * tau / BS
    base = np.where(c <= 32, np.cos(ang), np.sin(ang))
    scale = np.where((c % 32) == 0, 1.0 / BS, 2.0 / BS)
    G = base * scale
    bd = np.zeros((128, 128), np.float32)
    bd[:64, :64] = G
    bd[64:, 64:] = G
    return bd.astype(ml_dtypes.bfloat16)


def make_wmats(W):
    W = np.asarray(W, np.float32)
    s = np.arange(BS)
    k = np.arange(33)
    ang = 2 * np.pi * k[:, None] * s[None, :] / BS
    wr = np.einsum("ijs,ks->ijk", W, np.cos(ang))
    wi = np.einsum("ijs,ks->ijk", W, np.sin(ang))
    M = np.zeros((NK, 128, 128), np.float32)
    icol = np.empty(64, np.int64)
    for i in range(64):
        a, par = divmod(i, 2)
        icol[i] = 64 * par + a
    for kk in range(NK):
        if kk == 0:
            W32 = wr[:, :, 32]
        Wr, Wi = wr[:, :, kk], wi[:, :, kk]
        for par_j in range(2):
            jrows = np.arange(32) * 2 + par_j
            rre = 64 * par_j + np.arange(32)
            rim = rre + 32
            for i in range(64):
                cre = icol[i]
                cim = cre + 32
                if kk == 0:
                    M[0, rre, cre] = wr[i, jrows, 0]
                    M[0, rim, cim] = W32[i, jrows]
                else:
                    M[kk, rre, cre] = Wr[i, jrows]
                    M[kk, rim, cre] = -Wi[i, jrows]
                    M[kk, rre, cim] = Wi[i, jrows]
                    M[kk, rim, cim] = Wr[i, jrows]
    return np.ascontiguousarray(M.transpose(1, 0, 2)).astype(ml_dtypes.bfloat16)


def prep_x(x):
    """[B, 4096] f32 -> per-core [128, WP, NA, 2] bf16.

    partition p = par*64 + t (j = 2a+par); free = (w-pair, a, w-parity)."""
    xr = np.asarray(x, np.float32).reshape(N_CORES, WP, 2, NA, 2, 64)
    xp = xr.transpose(0, 4, 5, 1, 3, 2)      # [c, par, t, wp, a, w2]
    return np.ascontiguousarray(xp).reshape(
        N_CORES, 128, WP, NA, 2).astype(ml_dtypes.bfloat16)


def post_y(ys):
    """per-core [128, WP, NA, 2] bf16 -> [B, 4096] f32; p = par*64 + tau,
    i = 2a+par."""
    y = np.stack(ys).astype(np.float32)      # [c, 128, WP, NA, 2]
    y = y.reshape(N_CORES, 2, 64, WP, NA, 2)  # [c, par, tau, wp, a, w2]
    y = y.transpose(0, 3, 5, 4, 1, 2)        # [c, wp, w2, a, par, tau]
    return np.ascontiguousarray(y).reshape(B, OUT)


# ---------------- device kernel ----------------

def build_nc(reps=1):
    nc = bacc.Bacc("TRN2", target_bir_lowering=False, debug=False,
                   num_devices=N_CORES, dynamic_dma_scratch_size=8192)
    x_in = nc.dram_tensor("x", [128, WP, NA, 2], BF16, kind="ExternalInput")
    fmat = nc.dram_tensor("fmat", [128, 128], BF16, kind="ExternalInput")
    gmat = nc.dram_tensor("gmat", [128, 128], BF16, kind="ExternalInput")
    wmat = nc.dram_tensor("wmat", [128, NK, 128], BF16, kind="ExternalInput")
    y_out = nc.dram_tensor("y", [128, WP, NA, 2], BF16, kind="ExternalOutput")

    with tile.TileContext(nc) as tc:
        with (
            tc.tile_pool(name="consts", bufs=1) as cpool,
            tc.tile_pool(name="px", bufs=2) as px,
            tc.tile_pool(name="pt", bufs=1) as pt,
            tc.tile_pool(name="psa", bufs=2, space="PSUM") as psa,
            tc.tile_pool(name="psd", bufs=2, space="PSUM") as psd,
        ):
            f_sb = cpool.tile([128, 128], BF16)
            g_sb = cpool.tile([128, 128], BF16)
            w_sb = cpool.tile([128, NK, 128], BF16)
            nc.sync.dma_start(f_sb[:], fmat[:])
            nc.sync.dma_start(g_sb[:], gmat[:])
            nc.sync.dma_start(w_sb[:], wmat[:])

            def alloc_x(r):
                xh = []
                for h in range(2):
                    xt = px.tile([128, WP // 2, NA, 2], BF16, tag="x",
                                 name=f"xt{h}")
                    nc.sync.dma_start(
                        xt[:], x_in[:, h * (WP // 2):(h + 1) * (WP // 2)])
                    xh.append(xt)
                return xh

            def emit_fwd(xh):
                s_sb = pt.tile([128, WP, NA, 2], BF16, tag="s", name="s_sb")
                # s2: wp = 2*b + s; [b, k, s, w2] so T1-out is 2-strided in
                # k and the mid stage reads 8B-contiguous (s, w2) runs
                s2 = pt.tile([128, WP // 2, NK, 2, 2], BF16, tag="s2",
                             name="s2")
                for wq in range(NWQ):
                    h, hw = wq // 2, (wq % 2) * WC
                    ws = slice(wq * WC, (wq + 1) * WC)
                    for base, nj in ((0, 3), (12, 1), (16, 3), (28, 1)):
                        if nj == 3:
                            ps = psa.tile([128, 3, WC, 4, 2], F32, tag="ps3",
                                          name="psf")
                            for j in range(3):
                                a0 = base + j * 4
                                nc.tensor.matmul(
                                    ps[:, j], f_sb[:],
                                    xh[h][:, hw:hw + WC, a0:a0 + 4, :])
                            nc.scalar.copy(
                                s_sb[:, ws, base:base + 12, :].rearrange(
                                    "p w (j a) c -> p j w a c", j=3),
                                ps[:])
                        else:
                            ps = psd.tile([128, WC, 4, 2], F32, tag="ps1",
                                          name="psf")
                            nc.tensor.matmul(
                                ps[:], f_sb[:],
                                xh[h][:, hw:hw + WC, base:base + 4, :])
                            nc.vector.tensor_copy(
                                s_sb[:, ws, base:base + 4, :], ps[:])
                return s_sb, s2

            def emit_t1(s_sb, s2):
                for wq in range(NWQ):
                    ws = slice(wq * WC, (wq + 1) * WC)
                    bs = slice(wq * (WC // 2), (wq + 1) * (WC // 2))
                    nc.vector.transpose(
                        s2[:, bs].bitcast(F32).rearrange(
                            "p b k s x -> p b s k x"),
                        s_sb[:, ws].bitcast(F32))

            def emit_mid(s2):
                o_sb = pt.tile([128, WP, NK, 2], BF16, tag="o", name="o_sb")
                for g4 in range(NK // 4):
                    k0 = g4 * 4
                    ps = psa.tile([128, 3, WP // 2, 2, 2], F32, tag="ps3",
                                  name="psm")
                    for j in range(3):
                        nc.tensor.matmul(
                            ps[:, j], w_sb[:, k0 + j, :],
                            s2[:, :, k0 + j, :, :])
                    nc.scalar.copy(
                        o_sb[:, :, k0:k0 + 3, :].rearrange(
                            "p (b s) k c -> p k b s c", s=2),
                        ps[:])
                    ps1 = psd.tile([128, WP // 2, 2, 2], F32, tag="ps1",
                                   name="psm1")
                    nc.tensor.matmul(
                        ps1[:], w_sb[:, k0 + 3, :], s2[:, :, k0 + 3, :, :])
                    nc.vector.tensor_copy(
                        o_sb[:, :, k0 + 3, :].rearrange(
                            "p (b s) c -> p b s c", s=2),
                        ps1[:])
                return o_sb

            def emit_t2(o_sb):
                v_sb = pt.tile([128, WP, NA, 2], BF16, tag="v", name="v_sb")
                for wq in range(NWQ):
                    ws = slice(wq * WC, (wq + 1) * WC)
                    nc.vector.transpose(
                        v_sb[:, ws].bitcast(F32), o_sb[:, ws].bitcast(F32))
                return v_sb

            def emit_inv(v_sb, pending_out):
                y_sb = pt.tile([128, WP, NA, 2], BF16, tag="y", name="y_sb")
                for wq in range(NWQ):
                    ws = slice(wq * WC, (wq + 1) * WC)
                    for base, nj in ((0, 3), (12, 1), (16, 3), (28, 1)):
                        if nj == 3:
                            ps = psa.tile([128, 3, WC, 4, 2], F32, tag="ps3",
                                          name="psi")
                            for j in range(3):
                                a0 = base + j * 4
                                nc.tensor.matmul(
                                    ps[:, j], g_sb[:],
                                    v_sb[:, ws, a0:a0 + 4, :])
                            nc.scalar.copy(
                                y_sb[:, ws, base:base + 12, :].rearrange(
                                    "p w (j a) c -> p j w a c", j=3),
                                ps[:])
                        else:
                            ps = psd.tile([128, WC, 4, 2], F32, tag="ps1",
                                          name="psi")
                            nc.tensor.matmul(
                                ps[:], g_sb[:],
                                v_sb[:, ws, base:base + 4, :])
                            nc.vector.tensor_copy(
                                y_sb[:, ws, base:base + 4, :], ps[:])
                    pending_out.append((y_out[:, ws], y_sb[:, ws]))

            # software pipeline: PE order [mid(r), fwd(r+1), inv(r)];
            # DVE order [mid-casts(r), T2(r), fwd-casts+T1(r+1), inv-casts(r)]
            pending_out = []
            xh = alloc_x(0)
            s_sb, s2 = emit_fwd(xh)
            emit_t1(s_sb, s2)
            for r in range(reps):
                if r + 1 < reps:
                    xh = alloc_x(r + 1)
                o_sb = emit_mid(s2)
                v_sb = emit_t2(o_sb)
                if r + 1 < reps:
                    s_sb, s2 = emit_fwd(xh)
                emit_inv(v_sb, pending_out)
                if r + 1 < reps:
                    emit_t1(s_sb, s2)
                for dst, src in pending_out:
                    nc.sync.dma_start(dst, src)
                pending_out = []

    nc.compile()
    return nc


_NC_CACHE = {}


def _in_maps(x, W):
    fmat = make_fmat()
    gmat = make_gmat()
    wmat = make_wmats(W)
    xp = prep_x(x)
    return [
        {"x": xp[c], "fmat": fmat, "gmat": gmat, "wmat": wmat}
        for c in range(N_CORES)
    ]


def run(x, W, reps=1):
    if reps not in _NC_CACHE:
        _NC_CACHE[reps] = build_nc(reps)
    nc = _NC_CACHE[reps]
    res = run_bass_kernel_spmd(nc, _in_maps(x, W), list(range(N_CORES)))
    return post_y([res.results[c]["y"] for c in range(N_CORES)])


def kernel(x, W):
    if 1 not in _NC_CACHE:
        _NC_CACHE[1] = build_nc(reps=1)
    res = run_bass_kernel_spmd(nc=_NC_CACHE[1], in_maps=_in_maps(x, W),
                               core_ids=list(range(N_CORES)))
    return post_y([res.results[c]["y"] for c in range(N_CORES)])
